# revision 5
# baseline (speedup 1.0000x reference)
"""GCMC layer on trn2 — dma_gather + one-hot PE segment-sum implementation.

Design (per core, dst-sharded: core c owns dst nodes [c*NSH, (c+1)*NSH)):
  - fsrc table [2*NU, 64] in DRAM = feat * cj for both sides (built on device).
  - Per (dir, rating) stream: edges with dst in shard, grouped host-side by
    (src-slab, dst-tile). Slabs are 32768-row windows of the table so gather
    indices fit int16 (dma_gather requirement). Each (slab, tile) segment is
    padded to a multiple of 128 edges (chunk). Segment sizes are maxed across
    cores so the program is SPMD-uniform; padding slots gather garbage rows
    but carry dstloc = -1 which the one-hot kills.
  - dma_gather (SWDGE CounterMachine path, ~9ns/row vs ~1.6us/row for generic
    indirect DMA) pulls 256B rows into xb tiles, edge j at [j%128, j//128].
  - Segment-sum on the PE: per 128-edge chunk, one-hot O[e, d] =
    (dstloc[e] == iota[d]) built by one DVE is_equal per 16 chunks;
    psum[d, :] += O^T @ X accumulates a dst-tile across its chunks, then a
    DVE add flushes into the SBUF-resident Y_acc [128, R*TP*64].
  - Transform per (dir, tile): per-r PE transpose + matmul with W_r
    (basis-combined), relu(msg * ci) on ACT, PE transpose + fc matmul with
    bias, dense store. No indirect DMA anywhere except the gathers.
Host only bins/sorts indices and packs int16 index blocks.
"""
import sys
import numpy as np

sys.path.insert(0, '/opt/trn_rl_repo')

import concourse.bass as bass
import concourse.tile as tile
import concourse.mybir as mybir
from concourse import bacc
from concourse.masks import make_identity

F32 = mybir.dt.float32
I16 = mybir.dt.int16
ALU = mybir.AluOpType
ACTF = mybir.ActivationFunctionType
P = 128


class Cfg:
    def __init__(self, NU=100000, NM=100000, R=5, E=1000000, IN=64, BAS=4,
                 NCORES=8):
        assert NU == NM
        self.NU, self.NM, self.R, self.E, self.IN, self.BAS = NU, NM, R, E, IN, BAS
        self.MPR = 16
        self.MSG = self.MPR * R
        self.OUT = 64
        self.NCORES = NCORES
        self.NSH = NU // NCORES               # dsts per core
        self.TP = -(-self.NSH // P)           # dst tiles per direction
        self.SLAB = 32768
        self.NSLAB = -(-NU // self.SLAB)
        self.CALL_CH = 64                     # chunks (128 rows) per gather call
        self.OHC = 16                         # chunks per one-hot DVE block


# ----------------------------------------------------------------- host prep

def build_plan(cfg, edge_user, edge_movie):
    """Group edges per (dir, r, core) by (src-slab, dst-tile); pad each
    segment to the max chunk count across cores (SPMD-uniform structure).

    Returns:
      streams: per (d, r): dict(calls=[dict(slab, segs=[(t, ch)], rows,
               c16ofs, ccofs)], rows)
      gidx:  per-core int16 [128, G16] gather-index blocks
      dloc:  per-core f32  [128, GCC] in-tile dst ids (-1 = padding)
    """
    NC, NSH, TP, R = cfg.NCORES, cfg.NSH, cfg.TP, cfg.R
    NSLAB, SLAB = cfg.NSLAB, cfg.SLAB
    eu = np.asarray(edge_user)
    em = np.asarray(edge_movie)

    streams = []
    gblocks = [[] for _ in range(NC)]
    dblocks = [[] for _ in range(NC)]
    c16ofs = 0
    ccofs = 0
    for d in range(2):
        src_all, dst_all = (eu, em) if d == 0 else (em, eu)
        side = 0 if d == 0 else 1
        for r in range(R):
            src, dst = src_all[r], dst_all[r]
            core = dst // NSH
            pc = []
            cnt = np.zeros((NC, NSLAB, TP), np.int64)
            for c in range(NC):
                m = core == c
                s = src[m].astype(np.int64)
                ld = (dst[m] - c * NSH).astype(np.int64)
                slab = s >> 15
                t = ld >> 7
                np.add.at(cnt[c], (slab, t), 1)
                pc.append((s, ld, slab, t))
            ch = -(-(cnt.max(axis=0)) // P)          # [NSLAB, TP] chunks

            # greedy call packing over (slab, tile) segments
            calls = []
            cur = None
            for sb in range(NSLAB):
                for t in range(TP):
                    n = int(ch[sb, t])
                    if n == 0:
                        continue
                    if cur is None or cur['slab'] != sb or \
                       cur['ch'] + n > cfg.CALL_CH:
                        if cur is not None:
                            calls.append(cur)
                        cur = dict(slab=sb, segs=[], ch=0)
                    cur['segs'].append((t, n))
                    cur['ch'] += n
            if cur is not None:
                calls.append(cur)

            # per-segment padded row offsets (same for all cores)
            rows_tot = int(ch.sum()) * P
            seg_ofs = {}
            o = 0
            for cl in calls:
                cl['rows'] = cl['ch'] * P
                cl['rowofs'] = o
                for t, n in cl['segs']:
                    seg_ofs[(cl['slab'], t)] = o
                    o += n * P

            # per-core padded streams
            for c in range(NC):
                s, ld, slab, t = pc[c]
                order = np.lexsort((ld, slab))
                s, ld, slab, t = s[order], ld[order], slab[order], t[order]
                # rank within (slab, tile) group
                key = slab * TP + t
                bnd = np.flatnonzero(np.diff(key, prepend=-1))
                rank = np.arange(key.size) - np.repeat(
                    bnd, np.diff(np.append(bnd, key.size)))
                base = np.array([seg_ofs.get((int(sb), int(tt)), -1)
                                 for sb, tt in zip(slab[bnd], t[bnd])])
                pos = np.repeat(base, np.diff(np.append(bnd, key.size))) + rank
                gs = np.zeros(rows_tot, np.int16)
                dl = np.full(rows_tot, -1.0, np.float32)
                gs[pos] = (s & 32767).astype(np.int16)
                dl[pos] = (ld - (t << 7)).astype(np.float32)
                gb = np.tile(gs.reshape(-1, 16).T, (8, 1))      # [128, rows/16]
                db = np.ascontiguousarray(
                    dl.reshape(-1, P).T)                         # [128, rows/128]
                gblocks[c].append(gb)
                dblocks[c].append(db)

            for cl in calls:
                cl['c16ofs'] = c16ofs + cl['rowofs'] // 16
                cl['ccofs'] = ccofs + cl['rowofs'] // P
            streams.append(dict(d=d, r=r, side=side, calls=calls,
                                rows=rows_tot))
            c16ofs += rows_tot // 16
            ccofs += rows_tot // P

    gidx = [np.concatenate(g, axis=1) for g in gblocks]
    dloc = [np.concatenate(db, axis=1) for db in dblocks]
    return streams, gidx, dloc


# ------------------------------------------------------------- numpy model

def model(cfg, streams, gidx, dloc, inputs):
    """Numpy mirror of the device program, for validation."""
    R, NSH, TP, IN = cfg.R, cfg.NSH, cfg.TP, cfg.IN
    W = np.einsum('rb,bio->rio', np.asarray(inputs['att']),
                  np.asarray(inputs['basis'])).astype(np.float32)
    fc_w = np.asarray(inputs['fc_w'])
    fc_b = np.asarray(inputs['fc_b'])
    tbl = np.concatenate([
        np.asarray(inputs['ufeat']) * np.asarray(inputs['cj_user']),
        np.asarray(inputs['ifeat']) * np.asarray(inputs['cj_movie'])],
        axis=0).astype(np.float32)
    ci = [np.asarray(inputs['ci_movie']), np.asarray(inputs['ci_user'])]

    u_out = np.zeros((cfg.NU, cfg.OUT), np.float32)
    m_out = np.zeros((cfg.NM, cfg.OUT), np.float32)
    for c in range(cfg.NCORES):
        for d in range(2):
            yacc = np.zeros((R, TP * P, IN), np.float32)
            for st in streams:
                if st['d'] != d:
                    continue
                r = st['r']
                for cl in st['calls']:
                    base = st['side'] * cfg.NU + cl['slab'] * cfg.SLAB
                    nrow = min(cfg.SLAB, cfg.NU - cl['slab'] * cfg.SLAB)
                    o = cl['rowofs']
                    gs = gidx[c][:, cl['c16ofs']:cl['c16ofs'] + cl['rows'] // 16]
                    dl = dloc[c][:, cl['ccofs']:cl['ccofs'] + cl['rows'] // P]
                    lin_g = gs[:16].T.reshape(-1)
                    lin_d = dl.T.reshape(-1)
                    x = tbl[base + lin_g.astype(np.int64)]
                    co = 0
                    for t, n in cl['segs']:
                        seg_d = lin_d[co:co + n * P]
                        seg_x = x[co:co + n * P]
                        msk = seg_d >= 0
                        np.add.at(yacc[r], t * P + seg_d[msk].astype(np.int64),
                                  seg_x[msk])
                        co += n * P
            msgs = np.zeros((TP * P, cfg.MSG), np.float32)
            for t in range(TP):
                for r in range(R):
                    msgs[t * P:(t + 1) * P, r * 16:(r + 1) * 16] = \
                        yacc[r, t * P:(t + 1) * P] @ W[r]
            cish = np.zeros((TP * P, 1), np.float32)
            cish[:NSH] = ci[d][c * NSH:(c + 1) * NSH]
            z = np.maximum(msgs * cish, 0.0) @ fc_w.T + fc_b
            if d == 0:
                m_out[c * NSH:(c + 1) * NSH] = z[:NSH]
            else:
                u_out[c * NSH:(c + 1) * NSH] = z[:NSH]
    return u_out, m_out


# ---------------------------------------------------------- device program

def build_program(cfg, streams, g16cols, gccols, num_devices):
    nc = bacc.Bacc("TRN2", target_bir_lowering=False, debug=False,
                   num_devices=num_devices)
    NU, IN, R, BAS = cfg.NU, cfg.IN, cfg.R, cfg.BAS
    TP = cfg.TP

    ufeat = nc.dram_tensor("ufeat", (NU, IN), F32, kind="ExternalInput")
    ifeat = nc.dram_tensor("ifeat", (NU, IN), F32, kind="ExternalInput")
    cj_u = nc.dram_tensor("cj_u", (NU, 1), F32, kind="ExternalInput")
    cj_m = nc.dram_tensor("cj_m", (NU, 1), F32, kind="ExternalInput")
    ci_sh = nc.dram_tensor("ci_sh", (2 * TP * P, 1), F32, kind="ExternalInput")
    attT = nc.dram_tensor("attT", (BAS, R), F32, kind="ExternalInput")
    basis2 = nc.dram_tensor("basis2", (BAS, IN * 16), F32, kind="ExternalInput")
    fc_w = nc.dram_tensor("fc_w", (64, cfg.MSG), F32, kind="ExternalInput")
    fc_b = nc.dram_tensor("fc_b", (1, 64), F32, kind="ExternalInput")
    iota_d = nc.dram_tensor("iota128", (P, P), F32, kind="ExternalInput")
    gidx = nc.dram_tensor("gidx", (P, g16cols), I16, kind="ExternalInput")
    dloc = nc.dram_tensor("dloc", (P, gccols), F32, kind="ExternalInput")

    m_out = nc.dram_tensor("m_out", (TP * P, 64), F32, kind="ExternalOutput")
    u_out = nc.dram_tensor("u_out", (TP * P, 64), F32, kind="ExternalOutput")

    fsrc = nc.dram_tensor("fsrc", (2 * NU, IN), F32, kind="Internal")
    wscr = nc.dram_tensor("wscr", (R, IN * 16), F32, kind="Internal")

    with tile.TileContext(nc) as tc:
        with tc.tile_pool(name="const", bufs=1) as pool:
            # ---------------- constants ----------------
            pp_ctx = tc.tile_pool(name="cpsum", bufs=2, space="PSUM")
            pp = pp_ctx.__enter__()
            ident = pool.tile([P, P], F32)
            make_identity(nc, ident[:])

            at = pool.tile([BAS, R], F32)
            bs = pool.tile([BAS, IN * 16], F32)
            nc.sync.dma_start(out=at[:], in_=attT.ap()[:])
            nc.sync.dma_start(out=bs[:], in_=basis2.ap()[:])
            w5 = pool.tile([R, IN * 16], F32)
            half = IN * 16 // 2
            for h in range(2):
                ps = pp.tile([R, half], F32, space="PSUM", tag="w5ps")
                nc.tensor.matmul(out=ps[:], lhsT=at[:],
                                 rhs=bs[:, h * half:(h + 1) * half],
                                 start=True, stop=True)
                nc.scalar.copy(out=w5[:, h * half:(h + 1) * half], in_=ps[:])
            nc.sync.dma_start(out=wscr.ap()[:], in_=w5[:])
            w64 = pool.tile([IN, R, 16], F32)
            nc.sync.dma_start(
                out=w64[:], in_=wscr.ap()[:].rearrange("r (k o) -> k r o", k=IN))

            fcw = pool.tile([64, cfg.MSG], F32)
            nc.sync.dma_start(out=fcw[:], in_=fc_w.ap()[:])
            psT = pp.tile([cfg.MSG, 64], F32, space="PSUM", tag="fcT")
            nc.tensor.transpose(out=psT[:], in_=fcw[:], identity=ident[:64, :64])
            fcwT = pool.tile([cfg.MSG, 64], F32)
            nc.scalar.copy(out=fcwT[:], in_=psT[:])
            fcb = pool.tile([P, 64], F32)
            nc.sync.dma_start(out=fcb[:], in_=fc_b.ap()[:].to_broadcast((P, 64)))

            cisb = pool.tile([P, 2 * TP], F32)
            nc.sync.dma_start(
                out=cisb[:],
                in_=ci_sh.ap()[:].rearrange("(t p) o -> p (t o)", p=P))
            iota = pool.tile([P, 1, P], F32)
            nc.sync.dma_start(out=iota[:, 0, :], in_=iota_d.ap()[:])
            pp_ctx.__exit__(None, None, None)

            # ---------------- table build ----------------
            with tc.tile_pool(name="p1", bufs=3) as p1:
                GT = 8
                for side, (feat, cj) in enumerate(((ufeat, cj_u), (ifeat, cj_m))):
                    starts = list(range(0, NU - GT * P + 1, GT * P))
                    if NU % (GT * P):
                        starts.append(NU - GT * P)
                    for g0 in starts:
                        ft = p1.tile([P, GT, IN], F32, tag="ft")
                        cjt = p1.tile([P, GT, 1], F32, tag="cj")
                        nc.sync.dma_start(
                            out=ft[:], in_=feat.ap()[g0:g0 + GT * P].rearrange(
                                "(p a) d -> p a d", p=P))
                        nc.sync.dma_start(
                            out=cjt[:], in_=cj.ap()[g0:g0 + GT * P].rearrange(
                                "(p a) d -> p a d", p=P))
                        sc = p1.tile([P, GT, IN], F32, tag="sc")
                        nc.vector.tensor_tensor(
                            out=sc[:], in0=ft[:],
                            in1=cjt[:].to_broadcast((P, GT, IN)),
                            op=ALU.mult)
                        nc.sync.dma_start(
                            out=fsrc.ap()[side * NU + g0: side * NU + g0 + GT * P]
                                .rearrange("(p a) d -> p a d", p=P),
                            in_=sc[:])

            # ---------------- gather + segment-sum + transform ----------------
            CC = cfg.CALL_CH
            with tc.tile_pool(name="yacc", bufs=1) as yp, \
                 tc.tile_pool(name="io", bufs=3) as iop, \
                 tc.tile_pool(name="xb", bufs=2) as xp, \
                 tc.tile_pool(name="oh", bufs=2) as ohp, \
                 tc.tile_pool(name="ps", bufs=3, space="PSUM") as psp, \
                 tc.tile_pool(name="p3", bufs=3) as p3, \
                 tc.tile_pool(name="p3ps", bufs=1, space="PSUM") as p3p:
                for d in range(2):
                    yacc = yp.tile([P, R * TP * IN], F32, tag="yacc")
                    nc.vector.memset(yacc[:], 0.0)
                    for st in streams:
                        if st['d'] != d:
                            continue
                        r = st['r']
                        for cl in st['calls']:
                            rows = cl['rows']
                            cc = rows // P
                            c16 = rows // 16
                            base = st['side'] * NU + cl['slab'] * cfg.SLAB
                            nrow = min(cfg.SLAB, NU - cl['slab'] * cfg.SLAB)
                            gt = iop.tile([P, CC * 8], I16, tag="gt")
                            nc.sync.dma_start(
                                out=gt[:, :c16],
                                in_=gidx.ap()[:, cl['c16ofs']:cl['c16ofs'] + c16])
                            dl = iop.tile([P, CC, 1], F32, tag="dl")
                            nc.sync.dma_start(
                                out=dl[:, :cc, :],
                                in_=dloc.ap()[:, cl['ccofs']:cl['ccofs'] + cc]
                                    .rearrange("p (c one) -> p c one", one=1))
                            xb = xp.tile([P, CC, IN], F32, tag="xb")
                            nc.gpsimd.dma_gather(
                                xb[:, :cc, :], fsrc.ap()[base:base + nrow],
                                gt[:, :c16], rows, rows, IN,
                                single_packet=False)
                            # chunk -> (tile, first, last)
                            chmeta = []
                            for t, n in cl['segs']:
                                for k in range(n):
                                    chmeta.append((t, k == 0, k == n - 1))
                            ps = None
                            for ob in range(0, cc, cfg.OHC):
                                obc = min(cfg.OHC, cc - ob)
                                oh = ohp.tile([P, cfg.OHC, P], F32, tag="oh")
                                nc.vector.tensor_tensor(
                                    out=oh[:, :obc, :],
                                    in0=dl[:, ob:ob + obc, :].to_broadcast(
                                        (P, obc, P)),
                                    in1=iota[:, 0:1, :].to_broadcast(
                                        (P, obc, P)),
                                    op=ALU.is_equal)
                                for j in range(obc):
                                    t, first, last = chmeta[ob + j]
                                    if first:
                                        ps = psp.tile([P, IN], F32,
                                                      space="PSUM", tag="ps")
                                    nc.tensor.matmul(
                                        out=ps[:], lhsT=oh[:, j, :],
                                        rhs=xb[:, ob + j, :],
                                        start=first, stop=last)
                                    if last:
                                        ys = yacc[:, (r * TP + t) * IN:
                                                  (r * TP + t + 1) * IN]
                                        nc.vector.tensor_tensor(
                                            out=ys, in0=ys, in1=ps[:],
                                            op=ALU.add)
                    # ---------------- transform ----------------
                    for t in range(TP):
                        msgp = p3p.tile([P, cfg.MSG], F32, space="PSUM",
                                        tag="msgp")
                        for r in range(R):
                            ys = yacc[:, (r * TP + t) * IN:(r * TP + t + 1) * IN]
                            pst = p3p.tile([IN, P], F32, space="PSUM", tag="pst")
                            nc.tensor.transpose(out=pst[:], in_=ys,
                                                identity=ident[:])
                            yT = p3.tile([IN, P], F32, tag="yT")
                            if r % 2 == 0:
                                nc.scalar.copy(out=yT[:], in_=pst[:])
                            else:
                                nc.vector.tensor_copy(out=yT[:], in_=pst[:])
                            nc.tensor.matmul(
                                out=msgp[:, r * 16:(r + 1) * 16],
                                lhsT=yT[:], rhs=w64[:, r, :],
                                start=True, stop=True)
                        msg = p3.tile([P, cfg.MSG], F32, tag="msg")
                        nc.scalar.activation(
                            out=msg[:], in_=msgp[:],
                            func=ACTF.Relu,
                            scale=cisb[:, d * TP + t: d * TP + t + 1])
                        psmT = p3p.tile([cfg.MSG, P], F32, space="PSUM",
                                        tag="psmT")
                        nc.tensor.transpose(out=psmT[:], in_=msg[:],
                                            identity=ident[:])
                        msgT = p3.tile([cfg.MSG, P], F32, tag="msgT")
                        nc.scalar.copy(out=msgT[:], in_=psmT[:])
                        fcp = p3p.tile([P, 64], F32, space="PSUM", tag="fcp")
                        nc.tensor.matmul(
                            out=fcp[:], lhsT=msgT[:], rhs=fcwT[:],
                            start=True, stop=True)
                        osb = p3.tile([P, 64], F32, tag="osb")
                        nc.vector.tensor_tensor(out=osb[:], in0=fcp[:],
                                                in1=fcb[:], op=ALU.add)
                        dst = m_out if d == 0 else u_out
                        nc.sync.dma_start(
                            out=dst.ap()[t * P:(t + 1) * P], in_=osb[:])
    nc.compile()
    return nc


# ----------------------------------------------------------------- kernel

def make_in_maps(cfg, gidx, dloc, inputs):
    ins = {k: np.asarray(v) for k, v in inputs.items()}
    iota = np.tile(np.arange(P, dtype=np.float32), (P, 1))
    base = dict(
        ufeat=ins['ufeat'], ifeat=ins['ifeat'],
        cj_u=ins['cj_user'], cj_m=ins['cj_movie'],
        attT=np.ascontiguousarray(ins['att'].T),
        basis2=ins['basis'].reshape(cfg.BAS, cfg.IN * 16).copy(),
        fc_w=ins['fc_w'], fc_b=ins['fc_b'].reshape(1, 64).copy(),
        iota128=iota,
    )
    in_maps = []
    for c in range(cfg.NCORES):
        ci = np.zeros((2 * cfg.TP * P, 1), np.float32)
        ci[:cfg.NSH] = ins['ci_movie'][c * cfg.NSH:(c + 1) * cfg.NSH]
        ci[cfg.TP * P:cfg.TP * P + cfg.NSH] = \
            ins['ci_user'][c * cfg.NSH:(c + 1) * cfg.NSH]
        in_maps.append({**base, 'ci_sh': ci, 'gidx': gidx[c], 'dloc': dloc[c]})
    return in_maps


def assemble(cfg, results):
    u = np.concatenate([results[c]['u_out'][:cfg.NSH]
                        for c in range(cfg.NCORES)])
    m = np.concatenate([results[c]['m_out'][:cfg.NSH]
                        for c in range(cfg.NCORES)])
    return u, m


def kernel(**inputs):
    from concourse import bass_utils
    cfg = Cfg()
    streams, gidx, dloc = build_plan(cfg, inputs['edge_user'],
                                     inputs['edge_movie'])
    nc = build_program(cfg, streams, gidx[0].shape[1], dloc[0].shape[1],
                       cfg.NCORES)
    in_maps = make_in_maps(cfg, gidx, dloc, inputs)
    res = bass_utils.run_bass_kernel_spmd(nc, in_maps,
                                          core_ids=list(range(cfg.NCORES)))
    return assemble(cfg, res.results)


# revision 7
# speedup vs baseline: 1.0746x; 1.0746x over previous
"""GCMC layer on trn2 — dma_gather + one-hot PE segment-sum implementation.

Design (per core, dst-sharded: core c owns dst nodes [c*NSH, (c+1)*NSH)):
  - fsrc table [2*NU, 64] in DRAM = feat * cj for both sides (built on device).
  - Per (dir, rating) stream: edges with dst in shard, grouped host-side by
    (src-slab, dst-tile). Slabs are 32768-row windows of the table so gather
    indices fit int16 (dma_gather requirement). Each (slab, tile) segment is
    padded to a multiple of 128 edges (chunk). Segment sizes are maxed across
    cores so the program is SPMD-uniform; padding slots gather garbage rows
    but carry dstloc = -1 which the one-hot kills.
  - dma_gather (SWDGE CounterMachine path, ~9ns/row vs ~1.6us/row for generic
    indirect DMA) pulls 256B rows into xb tiles, edge j at [j%128, j//128].
  - Segment-sum on the PE: per 128-edge chunk, one-hot O[e, d] =
    (dstloc[e] == iota[d]) built by one DVE is_equal per 16 chunks;
    psum[d, :] += O^T @ X accumulates a dst-tile across its chunks, then a
    DVE add flushes into the SBUF-resident Y_acc [128, R*TP*64].
  - Transform per (dir, tile): per-r PE transpose + matmul with W_r
    (basis-combined), relu(msg * ci) on ACT, PE transpose + fc matmul with
    bias, dense store. No indirect DMA anywhere except the gathers.
Host only bins/sorts indices and packs int16 index blocks.
"""
import sys
import numpy as np

sys.path.insert(0, '/opt/trn_rl_repo')

import concourse.bass as bass
import concourse.tile as tile
import concourse.mybir as mybir
from concourse import bacc
from concourse.masks import make_identity

F32 = mybir.dt.float32
BF16 = mybir.dt.bfloat16
I16 = mybir.dt.int16
ALU = mybir.AluOpType
ACTF = mybir.ActivationFunctionType
P = 128


class Cfg:
    def __init__(self, NU=100000, NM=100000, R=5, E=1000000, IN=64, BAS=4,
                 NCORES=8):
        assert NU == NM
        self.NU, self.NM, self.R, self.E, self.IN, self.BAS = NU, NM, R, E, IN, BAS
        self.MPR = 16
        self.MSG = self.MPR * R
        self.OUT = 64
        self.NCORES = NCORES
        self.NSH = NU // NCORES               # dsts per core
        self.TP = -(-self.NSH // P)           # dst tiles per direction
        self.SLAB = 32768
        self.NSLAB = -(-NU // self.SLAB)
        self.CALL_CH = 64                     # chunks (128 rows) per gather call
        self.OHC = 16                         # chunks per one-hot DVE block


# ----------------------------------------------------------------- host prep

def build_plan(cfg, edge_user, edge_movie):
    """Group edges per (dir, r, core) by (src-slab, dst-tile); pad each
    segment to the max chunk count across cores (SPMD-uniform structure).

    Returns:
      streams: per (d, r): dict(calls=[dict(slab, segs=[(t, ch)], rows,
               c16ofs, ccofs)], rows)
      gidx:  per-core int16 [128, G16] gather-index blocks
      dloc:  per-core f32  [128, GCC] in-tile dst ids (-1 = padding)
    """
    NC, NSH, TP, R = cfg.NCORES, cfg.NSH, cfg.TP, cfg.R
    NSLAB, SLAB = cfg.NSLAB, cfg.SLAB
    eu = np.asarray(edge_user)
    em = np.asarray(edge_movie)

    streams = []
    gblocks = [[] for _ in range(NC)]
    dblocks = [[] for _ in range(NC)]
    c16ofs = 0
    ccofs = 0
    for d in range(2):
        src_all, dst_all = (eu, em) if d == 0 else (em, eu)
        side = 0 if d == 0 else 1
        for r in range(R):
            src, dst = src_all[r], dst_all[r]
            core = dst // NSH
            pc = []
            cnt = np.zeros((NC, NSLAB, TP), np.int64)
            for c in range(NC):
                m = core == c
                s = src[m].astype(np.int64)
                ld = (dst[m] - c * NSH).astype(np.int64)
                slab = s >> 15
                t = ld >> 7
                np.add.at(cnt[c], (slab, t), 1)
                pc.append((s, ld, slab, t))
            ch = -(-(cnt.max(axis=0)) // P)          # [NSLAB, TP] chunks

            # greedy call packing over (slab, tile) segments
            calls = []
            cur = None
            for sb in range(NSLAB):
                for t in range(TP):
                    n = int(ch[sb, t])
                    if n == 0:
                        continue
                    if cur is None or cur['slab'] != sb or \
                       cur['ch'] + n > cfg.CALL_CH:
                        if cur is not None:
                            calls.append(cur)
                        cur = dict(slab=sb, segs=[], ch=0)
                    cur['segs'].append((t, n))
                    cur['ch'] += n
            if cur is not None:
                calls.append(cur)

            # per-segment padded row offsets (same for all cores)
            rows_tot = int(ch.sum()) * P
            seg_ofs = {}
            o = 0
            for cl in calls:
                cl['rows'] = cl['ch'] * P
                cl['rowofs'] = o
                for t, n in cl['segs']:
                    seg_ofs[(cl['slab'], t)] = o
                    o += n * P

            # per-core padded streams
            for c in range(NC):
                s, ld, slab, t = pc[c]
                order = np.lexsort((ld, slab))
                s, ld, slab, t = s[order], ld[order], slab[order], t[order]
                # rank within (slab, tile) group
                key = slab * TP + t
                bnd = np.flatnonzero(np.diff(key, prepend=-1))
                rank = np.arange(key.size) - np.repeat(
                    bnd, np.diff(np.append(bnd, key.size)))
                base = np.array([seg_ofs.get((int(sb), int(tt)), -1)
                                 for sb, tt in zip(slab[bnd], t[bnd])])
                pos = np.repeat(base, np.diff(np.append(bnd, key.size))) + rank
                gs = np.zeros(rows_tot, np.int16)
                dl = np.full(rows_tot, -1.0, np.float32)
                gs[pos] = (s & 32767).astype(np.int16)
                dl[pos] = (ld - (t << 7)).astype(np.float32)
                gb = np.tile(gs.reshape(-1, 16).T, (8, 1))      # [128, rows/16]
                db = np.ascontiguousarray(
                    dl.reshape(-1, P).T)                         # [128, rows/128]
                gblocks[c].append(gb)
                dblocks[c].append(db)

            for cl in calls:
                cl['c16ofs'] = c16ofs + cl['rowofs'] // 16
                cl['ccofs'] = ccofs + cl['rowofs'] // P
            streams.append(dict(d=d, r=r, side=side, calls=calls,
                                rows=rows_tot))
            c16ofs += rows_tot // 16
            ccofs += rows_tot // P

    gidx = [np.concatenate(g, axis=1) for g in gblocks]
    dloc = [np.concatenate(db, axis=1) for db in dblocks]
    return streams, gidx, dloc


# ------------------------------------------------------------- numpy model

def model(cfg, streams, gidx, dloc, inputs):
    """Numpy mirror of the device program, for validation."""
    R, NSH, TP, IN = cfg.R, cfg.NSH, cfg.TP, cfg.IN
    W = np.einsum('rb,bio->rio', np.asarray(inputs['att']),
                  np.asarray(inputs['basis'])).astype(np.float32)
    fc_w = np.asarray(inputs['fc_w'])
    fc_b = np.asarray(inputs['fc_b'])
    tbl = np.concatenate([
        np.asarray(inputs['ufeat']) * np.asarray(inputs['cj_user']),
        np.asarray(inputs['ifeat']) * np.asarray(inputs['cj_movie'])],
        axis=0).astype(np.float32)
    ci = [np.asarray(inputs['ci_movie']), np.asarray(inputs['ci_user'])]

    u_out = np.zeros((cfg.NU, cfg.OUT), np.float32)
    m_out = np.zeros((cfg.NM, cfg.OUT), np.float32)
    for c in range(cfg.NCORES):
        for d in range(2):
            yacc = np.zeros((R, TP * P, IN), np.float32)
            for st in streams:
                if st['d'] != d:
                    continue
                r = st['r']
                for cl in st['calls']:
                    base = st['side'] * cfg.NU + cl['slab'] * cfg.SLAB
                    nrow = min(cfg.SLAB, cfg.NU - cl['slab'] * cfg.SLAB)
                    o = cl['rowofs']
                    gs = gidx[c][:, cl['c16ofs']:cl['c16ofs'] + cl['rows'] // 16]
                    dl = dloc[c][:, cl['ccofs']:cl['ccofs'] + cl['rows'] // P]
                    lin_g = gs[:16].T.reshape(-1)
                    lin_d = dl.T.reshape(-1)
                    x = tbl[base + lin_g.astype(np.int64)]
                    co = 0
                    for t, n in cl['segs']:
                        seg_d = lin_d[co:co + n * P]
                        seg_x = x[co:co + n * P]
                        msk = seg_d >= 0
                        np.add.at(yacc[r], t * P + seg_d[msk].astype(np.int64),
                                  seg_x[msk])
                        co += n * P
            msgs = np.zeros((TP * P, cfg.MSG), np.float32)
            for t in range(TP):
                for r in range(R):
                    msgs[t * P:(t + 1) * P, r * 16:(r + 1) * 16] = \
                        yacc[r, t * P:(t + 1) * P] @ W[r]
            cish = np.zeros((TP * P, 1), np.float32)
            cish[:NSH] = ci[d][c * NSH:(c + 1) * NSH]
            z = np.maximum(msgs * cish, 0.0) @ fc_w.T + fc_b
            if d == 0:
                m_out[c * NSH:(c + 1) * NSH] = z[:NSH]
            else:
                u_out[c * NSH:(c + 1) * NSH] = z[:NSH]
    return u_out, m_out


# ---------------------------------------------------------- device program

def build_program(cfg, streams, g16cols, gccols, num_devices):
    nc = bacc.Bacc("TRN2", target_bir_lowering=False, debug=False,
                   num_devices=num_devices)
    NU, IN, R, BAS = cfg.NU, cfg.IN, cfg.R, cfg.BAS
    TP = cfg.TP

    ufeat = nc.dram_tensor("ufeat", (NU, IN), F32, kind="ExternalInput")
    ifeat = nc.dram_tensor("ifeat", (NU, IN), F32, kind="ExternalInput")
    cj_u = nc.dram_tensor("cj_u", (NU, 1), F32, kind="ExternalInput")
    cj_m = nc.dram_tensor("cj_m", (NU, 1), F32, kind="ExternalInput")
    ci_sh = nc.dram_tensor("ci_sh", (2 * TP * P, 1), F32, kind="ExternalInput")
    attT = nc.dram_tensor("attT", (BAS, R), F32, kind="ExternalInput")
    basis2 = nc.dram_tensor("basis2", (BAS, IN * 16), F32, kind="ExternalInput")
    fc_w = nc.dram_tensor("fc_w", (64, cfg.MSG), F32, kind="ExternalInput")
    fc_b = nc.dram_tensor("fc_b", (1, 64), F32, kind="ExternalInput")
    iota_d = nc.dram_tensor("iota128", (P, P), F32, kind="ExternalInput")
    gidx = nc.dram_tensor("gidx", (P, g16cols), I16, kind="ExternalInput")
    dloc = nc.dram_tensor("dloc", (P, gccols), F32, kind="ExternalInput")

    m_out = nc.dram_tensor("m_out", (TP * P, 64), F32, kind="ExternalOutput")
    u_out = nc.dram_tensor("u_out", (TP * P, 64), F32, kind="ExternalOutput")

    fsrc = nc.dram_tensor("fsrc", (2 * NU, IN), F32, kind="Internal")
    wscr = nc.dram_tensor("wscr", (R, IN * 16), F32, kind="Internal")

    with tile.TileContext(nc) as tc:
        with tc.tile_pool(name="const", bufs=1) as pool:
            # ---------------- constants ----------------
            pp_ctx = tc.tile_pool(name="cpsum", bufs=2, space="PSUM")
            pp = pp_ctx.__enter__()
            ident = pool.tile([P, P], F32)
            make_identity(nc, ident[:])
            ident16 = pool.tile([P, P], BF16)
            make_identity(nc, ident16[:])

            at = pool.tile([BAS, R], F32)
            bs = pool.tile([BAS, IN * 16], F32)
            nc.sync.dma_start(out=at[:], in_=attT.ap()[:])
            nc.sync.dma_start(out=bs[:], in_=basis2.ap()[:])
            w5 = pool.tile([R, IN * 16], F32)
            half = IN * 16 // 2
            for h in range(2):
                ps = pp.tile([R, half], F32, space="PSUM", tag="w5ps")
                nc.tensor.matmul(out=ps[:], lhsT=at[:],
                                 rhs=bs[:, h * half:(h + 1) * half],
                                 start=True, stop=True)
                nc.scalar.copy(out=w5[:, h * half:(h + 1) * half], in_=ps[:])
            nc.sync.dma_start(out=wscr.ap()[:], in_=w5[:])
            w64 = pool.tile([IN, R, 16], F32)
            nc.sync.dma_start(
                out=w64[:], in_=wscr.ap()[:].rearrange("r (k o) -> k r o", k=IN))

            fcw = pool.tile([64, cfg.MSG], F32)
            nc.sync.dma_start(out=fcw[:], in_=fc_w.ap()[:])
            psT = pp.tile([cfg.MSG, 64], F32, space="PSUM", tag="fcT")
            nc.tensor.transpose(out=psT[:], in_=fcw[:], identity=ident[:64, :64])
            fcwT = pool.tile([cfg.MSG, 64], F32)
            nc.scalar.copy(out=fcwT[:], in_=psT[:])
            fcb = pool.tile([P, 64], F32)
            nc.sync.dma_start(out=fcb[:], in_=fc_b.ap()[:].to_broadcast((P, 64)))

            cisb = pool.tile([P, 2 * TP], F32)
            nc.sync.dma_start(
                out=cisb[:],
                in_=ci_sh.ap()[:].rearrange("(t p) o -> p (t o)", p=P))
            iota = pool.tile([P, 1, P], F32)
            nc.sync.dma_start(out=iota[:, 0, :], in_=iota_d.ap()[:])
            pp_ctx.__exit__(None, None, None)

            # ---------------- table build ----------------
            with tc.tile_pool(name="p1", bufs=3) as p1:
                GT = 8
                for side, (feat, cj) in enumerate(((ufeat, cj_u), (ifeat, cj_m))):
                    starts = list(range(0, NU - GT * P + 1, GT * P))
                    if NU % (GT * P):
                        starts.append(NU - GT * P)
                    for g0 in starts:
                        ft = p1.tile([P, GT, IN], F32, tag="ft")
                        cjt = p1.tile([P, GT, 1], F32, tag="cj")
                        nc.sync.dma_start(
                            out=ft[:], in_=feat.ap()[g0:g0 + GT * P].rearrange(
                                "(p a) d -> p a d", p=P))
                        nc.sync.dma_start(
                            out=cjt[:], in_=cj.ap()[g0:g0 + GT * P].rearrange(
                                "(p a) d -> p a d", p=P))
                        sc = p1.tile([P, GT, IN], F32, tag="sc")
                        nc.vector.tensor_tensor(
                            out=sc[:], in0=ft[:],
                            in1=cjt[:].to_broadcast((P, GT, IN)),
                            op=ALU.mult)
                        nc.sync.dma_start(
                            out=fsrc.ap()[side * NU + g0: side * NU + g0 + GT * P]
                                .rearrange("(p a) d -> p a d", p=P),
                            in_=sc[:])

            # ---------------- gather + segment-sum + transform ----------------
            CC = cfg.CALL_CH
            with tc.tile_pool(name="yacc", bufs=2) as yp, \
                 tc.tile_pool(name="io", bufs=3) as iop, \
                 tc.tile_pool(name="xb", bufs=3) as xp, \
                 tc.tile_pool(name="oh", bufs=2) as ohp, \
                 tc.tile_pool(name="ps", bufs=3, space="PSUM") as psp, \
                 tc.tile_pool(name="p3", bufs=3) as p3, \
                 tc.tile_pool(name="p3ps", bufs=1, space="PSUM") as p3p:
                for d in range(2):
                    yacc = yp.tile([P, R * TP * IN], BF16, tag="yacc")
                    nc.vector.memset(yacc[:], 0.0)
                    for st in streams:
                        if st['d'] != d:
                            continue
                        r = st['r']
                        for cl in st['calls']:
                            rows = cl['rows']
                            cc = rows // P
                            c16 = rows // 16
                            base = st['side'] * NU + cl['slab'] * cfg.SLAB
                            nrow = min(cfg.SLAB, NU - cl['slab'] * cfg.SLAB)
                            gt = iop.tile([P, CC * 8], I16, tag="gt")
                            nc.sync.dma_start(
                                out=gt[:, :c16],
                                in_=gidx.ap()[:, cl['c16ofs']:cl['c16ofs'] + c16])
                            dl = iop.tile([P, CC, 1], F32, tag="dl")
                            nc.sync.dma_start(
                                out=dl[:, :cc, :],
                                in_=dloc.ap()[:, cl['ccofs']:cl['ccofs'] + cc]
                                    .rearrange("p (c one) -> p c one", one=1))
                            xb = xp.tile([P, CC, IN], F32, tag="xb")
                            nc.gpsimd.dma_gather(
                                xb[:, :cc, :], fsrc.ap()[base:base + nrow],
                                gt[:, :c16], rows, rows, IN,
                                single_packet=False)
                            # chunk -> (tile, first, last)
                            chmeta = []
                            for t, n in cl['segs']:
                                for k in range(n):
                                    chmeta.append((t, k == 0, k == n - 1))
                            ps = None
                            for ob in range(0, cc, cfg.OHC):
                                obc = min(cfg.OHC, cc - ob)
                                oh = ohp.tile([P, cfg.OHC, P], F32, tag="oh")
                                nc.vector.tensor_tensor(
                                    out=oh[:, :obc, :],
                                    in0=dl[:, ob:ob + obc, :].to_broadcast(
                                        (P, obc, P)),
                                    in1=iota[:, 0:1, :].to_broadcast(
                                        (P, obc, P)),
                                    op=ALU.is_equal)
                                for j in range(obc):
                                    t, first, last = chmeta[ob + j]
                                    if first:
                                        ps = psp.tile([P, IN], F32,
                                                      space="PSUM", tag="ps")
                                    nc.tensor.matmul(
                                        out=ps[:], lhsT=oh[:, j, :],
                                        rhs=xb[:, ob + j, :],
                                        start=first, stop=last)
                                    if last:
                                        ys = yacc[:, (r * TP + t) * IN:
                                                  (r * TP + t + 1) * IN]
                                        nc.vector.tensor_tensor(
                                            out=ys, in0=ys, in1=ps[:],
                                            op=ALU.add)
                    # ---------------- transform ----------------
                    for t in range(TP):
                        msgp = p3p.tile([P, cfg.MSG], F32, space="PSUM",
                                        tag="msgp")
                        for r in range(R):
                            ys = yacc[:, (r * TP + t) * IN:(r * TP + t + 1) * IN]
                            pst = p3p.tile([IN, P], BF16, space="PSUM", tag="pst")
                            nc.tensor.transpose(out=pst[:], in_=ys,
                                                identity=ident16[:])
                            yT = p3.tile([IN, P], F32, tag="yT")
                            if r % 2 == 0:
                                nc.scalar.copy(out=yT[:], in_=pst[:])
                            else:
                                nc.vector.tensor_copy(out=yT[:], in_=pst[:])
                            nc.tensor.matmul(
                                out=msgp[:, r * 16:(r + 1) * 16],
                                lhsT=yT[:], rhs=w64[:, r, :],
                                start=True, stop=True)
                        msg = p3.tile([P, cfg.MSG], F32, tag="msg")
                        nc.scalar.activation(
                            out=msg[:], in_=msgp[:],
                            func=ACTF.Relu,
                            scale=cisb[:, d * TP + t: d * TP + t + 1])
                        psmT = p3p.tile([cfg.MSG, P], F32, space="PSUM",
                                        tag="psmT")
                        nc.tensor.transpose(out=psmT[:], in_=msg[:],
                                            identity=ident[:])
                        msgT = p3.tile([cfg.MSG, P], F32, tag="msgT")
                        nc.scalar.copy(out=msgT[:], in_=psmT[:])
                        fcp = p3p.tile([P, 64], F32, space="PSUM", tag="fcp")
                        nc.tensor.matmul(
                            out=fcp[:], lhsT=msgT[:], rhs=fcwT[:],
                            start=True, stop=True)
                        osb = p3.tile([P, 64], F32, tag="osb")
                        nc.vector.tensor_tensor(out=osb[:], in0=fcp[:],
                                                in1=fcb[:], op=ALU.add)
                        dst = m_out if d == 0 else u_out
                        nc.sync.dma_start(
                            out=dst.ap()[t * P:(t + 1) * P], in_=osb[:])
    nc.compile()
    return nc


# ----------------------------------------------------------------- kernel

def make_in_maps(cfg, gidx, dloc, inputs):
    ins = {k: np.asarray(v) for k, v in inputs.items()}
    iota = np.tile(np.arange(P, dtype=np.float32), (P, 1))
    base = dict(
        ufeat=ins['ufeat'], ifeat=ins['ifeat'],
        cj_u=ins['cj_user'], cj_m=ins['cj_movie'],
        attT=np.ascontiguousarray(ins['att'].T),
        basis2=ins['basis'].reshape(cfg.BAS, cfg.IN * 16).copy(),
        fc_w=ins['fc_w'], fc_b=ins['fc_b'].reshape(1, 64).copy(),
        iota128=iota,
    )
    in_maps = []
    for c in range(cfg.NCORES):
        ci = np.zeros((2 * cfg.TP * P, 1), np.float32)
        ci[:cfg.NSH] = ins['ci_movie'][c * cfg.NSH:(c + 1) * cfg.NSH]
        ci[cfg.TP * P:cfg.TP * P + cfg.NSH] = \
            ins['ci_user'][c * cfg.NSH:(c + 1) * cfg.NSH]
        in_maps.append({**base, 'ci_sh': ci, 'gidx': gidx[c], 'dloc': dloc[c]})
    return in_maps


def assemble(cfg, results):
    u = np.concatenate([results[c]['u_out'][:cfg.NSH]
                        for c in range(cfg.NCORES)])
    m = np.concatenate([results[c]['m_out'][:cfg.NSH]
                        for c in range(cfg.NCORES)])
    return u, m


def kernel(**inputs):
    from concourse import bass_utils
    cfg = Cfg()
    streams, gidx, dloc = build_plan(cfg, inputs['edge_user'],
                                     inputs['edge_movie'])
    nc = build_program(cfg, streams, gidx[0].shape[1], dloc[0].shape[1],
                       cfg.NCORES)
    in_maps = make_in_maps(cfg, gidx, dloc, inputs)
    res = bass_utils.run_bass_kernel_spmd(nc, in_maps,
                                          core_ids=list(range(cfg.NCORES)))
    return assemble(cfg, res.results)


# revision 11
# speedup vs baseline: 1.2298x; 1.1445x over previous
"""GCMC layer on trn2 — dma_gather + one-hot PE segment-sum implementation.

Design (per core, dst-sharded: core c owns dst nodes [c*NSH, (c+1)*NSH)):
  - fsrc table [2*NU, 64] in DRAM = feat * cj for both sides (built on device).
  - Per (dir, rating) stream: edges with dst in shard, grouped host-side by
    (src-slab, dst-tile). Slabs are 32768-row windows of the table so gather
    indices fit int16 (dma_gather requirement). Each (slab, tile) segment is
    padded to a multiple of 128 edges (chunk). Segment sizes are maxed across
    cores so the program is SPMD-uniform; padding slots gather garbage rows
    but carry dstloc = -1 which the one-hot kills.
  - dma_gather (SWDGE CounterMachine path, ~9ns/row vs ~1.6us/row for generic
    indirect DMA) pulls 256B rows into xb tiles, edge j at [j%128, j//128].
  - Segment-sum on the PE: per 128-edge chunk, one-hot O[e, d] =
    (dstloc[e] == iota[d]) built by one DVE is_equal per 16 chunks;
    psum[d, :] += O^T @ X accumulates a dst-tile across its chunks, then a
    DVE add flushes into the SBUF-resident Y_acc [128, R*TP*64].
  - Transform per (dir, tile): per-r PE transpose + matmul with W_r
    (basis-combined), relu(msg * ci) on ACT, PE transpose + fc matmul with
    bias, dense store. No indirect DMA anywhere except the gathers.
Host only bins/sorts indices and packs int16 index blocks.
"""
import sys
import numpy as np

sys.path.insert(0, '/opt/trn_rl_repo')

import concourse.bass as bass
import concourse.tile as tile
import concourse.mybir as mybir
from concourse import bacc
from concourse.masks import make_identity

F32 = mybir.dt.float32
BF16 = mybir.dt.bfloat16
I16 = mybir.dt.int16
ALU = mybir.AluOpType
ACTF = mybir.ActivationFunctionType
P = 128


class Cfg:
    def __init__(self, NU=100000, NM=100000, R=5, E=1000000, IN=64, BAS=4,
                 NCORES=8):
        assert NU == NM
        self.NU, self.NM, self.R, self.E, self.IN, self.BAS = NU, NM, R, E, IN, BAS
        self.MPR = 16
        self.MSG = self.MPR * R
        self.OUT = 64
        self.NCORES = NCORES
        self.NSH = NU // NCORES               # dsts per core
        self.TP = -(-self.NSH // P)           # dst tiles per direction
        self.SLAB = 32768
        self.NSLAB = -(-NU // self.SLAB)
        self.CALL_CH = 64                     # chunks (128 rows) per gather call
        self.OHC = 12                         # chunks per one-hot DVE block


# ----------------------------------------------------------------- host prep

def build_plan(cfg, edge_user, edge_movie):
    """Group edges per (dir, r, core) by (src-slab, dst-tile); pad each
    segment to the max chunk count across cores (SPMD-uniform structure).

    Returns:
      streams: per (d, r): dict(calls=[dict(slab, segs=[(t, ch)], rows,
               c16ofs, ccofs)], rows)
      gidx:  per-core int16 [128, G16] gather-index blocks
      dloc:  per-core f32  [128, GCC] in-tile dst ids (-1 = padding)
    """
    NC, NSH, TP, R = cfg.NCORES, cfg.NSH, cfg.TP, cfg.R
    NSLAB, SLAB = cfg.NSLAB, cfg.SLAB
    eu = np.asarray(edge_user)
    em = np.asarray(edge_movie)

    streams = []
    gblocks = [[] for _ in range(NC)]
    dblocks = [[] for _ in range(NC)]
    c16ofs = 0
    ccofs = 0
    for d in range(2):
        src_all, dst_all = (eu, em) if d == 0 else (em, eu)
        side = 0 if d == 0 else 1
        for r in range(R):
            src, dst = src_all[r], dst_all[r]
            core = dst // NSH
            pc = []
            cnt = np.zeros((NC, NSLAB, TP), np.int64)
            for c in range(NC):
                m = core == c
                s = src[m].astype(np.int64)
                ld = (dst[m] - c * NSH).astype(np.int64)
                slab = s >> 15
                t = ld >> 7
                np.add.at(cnt[c], (slab, t), 1)
                pc.append((s, ld, slab, t))
            ch = -(-(cnt.max(axis=0)) // P)          # [NSLAB, TP] chunks

            # greedy call packing over (slab, tile) segments
            calls = []
            cur = None
            for sb in range(NSLAB):
                for t in range(TP):
                    n = int(ch[sb, t])
                    if n == 0:
                        continue
                    if cur is None or cur['slab'] != sb or \
                       cur['ch'] + n > cfg.CALL_CH:
                        if cur is not None:
                            calls.append(cur)
                        cur = dict(slab=sb, segs=[], ch=0)
                    cur['segs'].append((t, n))
                    cur['ch'] += n
            if cur is not None:
                calls.append(cur)

            # per-segment padded row offsets (same for all cores)
            rows_tot = int(ch.sum()) * P
            seg_ofs = {}
            o = 0
            for cl in calls:
                cl['rows'] = cl['ch'] * P
                cl['rowofs'] = o
                for t, n in cl['segs']:
                    seg_ofs[(cl['slab'], t)] = o
                    o += n * P

            # per-core padded streams
            for c in range(NC):
                s, ld, slab, t = pc[c]
                order = np.lexsort((ld, slab))
                s, ld, slab, t = s[order], ld[order], slab[order], t[order]
                # rank within (slab, tile) group
                key = slab * TP + t
                bnd = np.flatnonzero(np.diff(key, prepend=-1))
                rank = np.arange(key.size) - np.repeat(
                    bnd, np.diff(np.append(bnd, key.size)))
                base = np.array([seg_ofs.get((int(sb), int(tt)), -1)
                                 for sb, tt in zip(slab[bnd], t[bnd])])
                pos = np.repeat(base, np.diff(np.append(bnd, key.size))) + rank
                gs = np.zeros(rows_tot, np.int16)
                dl = np.full(rows_tot, -1.0, np.float32)
                gs[pos] = (s & 32767).astype(np.int16)
                dl[pos] = (ld - (t << 7)).astype(np.float32)
                gb = np.tile(gs.reshape(-1, 16).T, (8, 1))      # [128, rows/16]
                db = np.ascontiguousarray(
                    dl.reshape(-1, P).T)                         # [128, rows/128]
                gblocks[c].append(gb)
                dblocks[c].append(db)

            for cl in calls:
                cl['c16ofs'] = c16ofs + cl['rowofs'] // 16
                cl['ccofs'] = ccofs + cl['rowofs'] // P
            streams.append(dict(d=d, r=r, side=side, calls=calls,
                                rows=rows_tot))
            c16ofs += rows_tot // 16
            ccofs += rows_tot // P

    gidx = [np.concatenate(g, axis=1) for g in gblocks]
    dloc = [np.concatenate(db, axis=1) for db in dblocks]
    return streams, gidx, dloc


# ------------------------------------------------------------- numpy model

def model(cfg, streams, gidx, dloc, inputs):
    """Numpy mirror of the device program, for validation."""
    R, NSH, TP, IN = cfg.R, cfg.NSH, cfg.TP, cfg.IN
    W = np.einsum('rb,bio->rio', np.asarray(inputs['att']),
                  np.asarray(inputs['basis'])).astype(np.float32)
    fc_w = np.asarray(inputs['fc_w'])
    fc_b = np.asarray(inputs['fc_b'])
    tbl = np.concatenate([
        np.asarray(inputs['ufeat']) * np.asarray(inputs['cj_user']),
        np.asarray(inputs['ifeat']) * np.asarray(inputs['cj_movie'])],
        axis=0).astype(np.float32)
    ci = [np.asarray(inputs['ci_movie']), np.asarray(inputs['ci_user'])]

    u_out = np.zeros((cfg.NU, cfg.OUT), np.float32)
    m_out = np.zeros((cfg.NM, cfg.OUT), np.float32)
    for c in range(cfg.NCORES):
        for d in range(2):
            yacc = np.zeros((R, TP * P, IN), np.float32)
            for st in streams:
                if st['d'] != d:
                    continue
                r = st['r']
                for cl in st['calls']:
                    base = st['side'] * cfg.NU + cl['slab'] * cfg.SLAB
                    nrow = min(cfg.SLAB, cfg.NU - cl['slab'] * cfg.SLAB)
                    o = cl['rowofs']
                    gs = gidx[c][:, cl['c16ofs']:cl['c16ofs'] + cl['rows'] // 16]
                    dl = dloc[c][:, cl['ccofs']:cl['ccofs'] + cl['rows'] // P]
                    lin_g = gs[:16].T.reshape(-1)
                    lin_d = dl.T.reshape(-1)
                    x = tbl[base + lin_g.astype(np.int64)]
                    co = 0
                    for t, n in cl['segs']:
                        seg_d = lin_d[co:co + n * P]
                        seg_x = x[co:co + n * P]
                        msk = seg_d >= 0
                        np.add.at(yacc[r], t * P + seg_d[msk].astype(np.int64),
                                  seg_x[msk])
                        co += n * P
            msgs = np.zeros((TP * P, cfg.MSG), np.float32)
            for t in range(TP):
                for r in range(R):
                    msgs[t * P:(t + 1) * P, r * 16:(r + 1) * 16] = \
                        yacc[r, t * P:(t + 1) * P] @ W[r]
            cish = np.zeros((TP * P, 1), np.float32)
            cish[:NSH] = ci[d][c * NSH:(c + 1) * NSH]
            z = np.maximum(msgs * cish, 0.0) @ fc_w.T + fc_b
            if d == 0:
                m_out[c * NSH:(c + 1) * NSH] = z[:NSH]
            else:
                u_out[c * NSH:(c + 1) * NSH] = z[:NSH]
    return u_out, m_out


# ---------------------------------------------------------- device program

def build_program(cfg, streams, g16cols, gccols, num_devices):
    nc = bacc.Bacc("TRN2", target_bir_lowering=False, debug=False,
                   num_devices=num_devices)
    NU, IN, R, BAS = cfg.NU, cfg.IN, cfg.R, cfg.BAS
    TP = cfg.TP

    ufeat = nc.dram_tensor("ufeat", (NU, IN), F32, kind="ExternalInput")
    ifeat = nc.dram_tensor("ifeat", (NU, IN), F32, kind="ExternalInput")
    cj_u = nc.dram_tensor("cj_u", (NU, 1), F32, kind="ExternalInput")
    cj_m = nc.dram_tensor("cj_m", (NU, 1), F32, kind="ExternalInput")
    ci_sh = nc.dram_tensor("ci_sh", (2 * TP * P, 1), F32, kind="ExternalInput")
    attT = nc.dram_tensor("attT", (BAS, R), F32, kind="ExternalInput")
    basis2 = nc.dram_tensor("basis2", (BAS, IN * 16), F32, kind="ExternalInput")
    fc_w = nc.dram_tensor("fc_w", (64, cfg.MSG), F32, kind="ExternalInput")
    fc_b = nc.dram_tensor("fc_b", (1, 64), F32, kind="ExternalInput")
    iota_d = nc.dram_tensor("iota128", (P, P), F32, kind="ExternalInput")
    gidx = nc.dram_tensor("gidx", (P, g16cols), I16, kind="ExternalInput")
    dloc = nc.dram_tensor("dloc", (P, gccols), F32, kind="ExternalInput")

    m_out = nc.dram_tensor("m_out", (TP * P, 64), F32, kind="ExternalOutput")
    u_out = nc.dram_tensor("u_out", (TP * P, 64), F32, kind="ExternalOutput")

    fsrc = nc.dram_tensor("fsrc", (2 * NU, IN), F32, kind="Internal")
    wscr = nc.dram_tensor("wscr", (R, IN * 16), F32, kind="Internal")

    with tile.TileContext(nc) as tc:
        with tc.tile_pool(name="const", bufs=1) as pool:
            # ---------------- constants ----------------
            pp_ctx = tc.tile_pool(name="cpsum", bufs=2, space="PSUM")
            pp = pp_ctx.__enter__()
            ident = pool.tile([P, P], F32)
            make_identity(nc, ident[:])
            ident16 = pool.tile([P, P], BF16)
            make_identity(nc, ident16[:])

            with tc.tile_pool(name="w0", bufs=1) as wp:
                at = wp.tile([BAS, R], F32)
                bs = wp.tile([BAS, IN * 16], F32)
                nc.sync.dma_start(out=at[:], in_=attT.ap()[:])
                nc.sync.dma_start(out=bs[:], in_=basis2.ap()[:])
                w5 = wp.tile([R, IN * 16], F32)
                half = IN * 16 // 2
                for h in range(2):
                    ps = pp.tile([R, half], F32, space="PSUM", tag="w5ps")
                    nc.tensor.matmul(out=ps[:], lhsT=at[:],
                                     rhs=bs[:, h * half:(h + 1) * half],
                                     start=True, stop=True)
                    nc.scalar.copy(out=w5[:, h * half:(h + 1) * half], in_=ps[:])
                nc.sync.dma_start(out=wscr.ap()[:], in_=w5[:])
            w64 = pool.tile([IN, R, 16], F32)
            nc.sync.dma_start(
                out=w64[:], in_=wscr.ap()[:].rearrange("r (k o) -> k r o", k=IN))
            w64b = pool.tile([IN, R, 16], BF16)
            nc.scalar.copy(out=w64b[:], in_=w64[:])

            fcw = pool.tile([64, cfg.MSG], F32)
            nc.sync.dma_start(out=fcw[:], in_=fc_w.ap()[:])
            psT = pp.tile([cfg.MSG, 64], F32, space="PSUM", tag="fcT")
            nc.tensor.transpose(out=psT[:], in_=fcw[:], identity=ident[:64, :64])
            fcwT = pool.tile([cfg.MSG, 64], F32)
            nc.scalar.copy(out=fcwT[:], in_=psT[:])
            fcb = pool.tile([P, 64], F32)
            nc.sync.dma_start(out=fcb[:], in_=fc_b.ap()[:].to_broadcast((P, 64)))

            cisb = pool.tile([P, 2 * TP], F32)
            nc.sync.dma_start(
                out=cisb[:],
                in_=ci_sh.ap()[:].rearrange("(t p) o -> p (t o)", p=P))
            iota = pool.tile([P, 1, P], F32)
            nc.sync.dma_start(out=iota[:, 0, :], in_=iota_d.ap()[:])
            pp_ctx.__exit__(None, None, None)

            # ---------------- table build ----------------
            with tc.tile_pool(name="p1", bufs=3) as p1:
                GT = 8
                for side, (feat, cj) in enumerate(((ufeat, cj_u), (ifeat, cj_m))):
                    starts = list(range(0, NU - GT * P + 1, GT * P))
                    if NU % (GT * P):
                        starts.append(NU - GT * P)
                    for g0 in starts:
                        ft = p1.tile([P, GT, IN], F32, tag="ft")
                        cjt = p1.tile([P, GT, 1], F32, tag="cj")
                        nc.sync.dma_start(
                            out=ft[:], in_=feat.ap()[g0:g0 + GT * P].rearrange(
                                "(p a) d -> p a d", p=P))
                        nc.sync.dma_start(
                            out=cjt[:], in_=cj.ap()[g0:g0 + GT * P].rearrange(
                                "(p a) d -> p a d", p=P))
                        sc = p1.tile([P, GT, IN], F32, tag="sc")
                        nc.vector.tensor_tensor(
                            out=sc[:], in0=ft[:],
                            in1=cjt[:].to_broadcast((P, GT, IN)),
                            op=ALU.mult)
                        nc.sync.dma_start(
                            out=fsrc.ap()[side * NU + g0: side * NU + g0 + GT * P]
                                .rearrange("(p a) d -> p a d", p=P),
                            in_=sc[:])

            # ---------------- gather + segment-sum + transform ----------------
            CC = cfg.CALL_CH
            with tc.tile_pool(name="yacc", bufs=2) as yp, \
                 tc.tile_pool(name="io", bufs=3) as iop, \
                 tc.tile_pool(name="xb", bufs=3) as xp, \
                 tc.tile_pool(name="xc", bufs=2) as xcp, \
                 tc.tile_pool(name="oh", bufs=2) as ohp, \
                 tc.tile_pool(name="ps", bufs=3, space="PSUM") as psp, \
                 tc.tile_pool(name="p3", bufs=3) as p3, \
                 tc.tile_pool(name="p3ps", bufs=1, space="PSUM") as p3p:
                for d in range(2):
                    yacc = yp.tile([P, R * TP * IN], BF16, tag="yacc")
                    nc.vector.memset(yacc[:], 0.0)
                    for st in streams:
                        if st['d'] != d:
                            continue
                        r = st['r']
                        for cl in st['calls']:
                            rows = cl['rows']
                            cc = rows // P
                            c16 = rows // 16
                            base = st['side'] * NU + cl['slab'] * cfg.SLAB
                            nrow = min(cfg.SLAB, NU - cl['slab'] * cfg.SLAB)
                            gt = iop.tile([P, CC * 8], I16, tag="gt")
                            nc.sync.dma_start(
                                out=gt[:, :c16],
                                in_=gidx.ap()[:, cl['c16ofs']:cl['c16ofs'] + c16])
                            dl = iop.tile([P, CC, 1], F32, tag="dl")
                            nc.sync.dma_start(
                                out=dl[:, :cc, :],
                                in_=dloc.ap()[:, cl['ccofs']:cl['ccofs'] + cc]
                                    .rearrange("p (c one) -> p c one", one=1))
                            xb = xp.tile([P, CC, IN], F32, tag="xb")
                            nc.gpsimd.dma_gather(
                                xb[:, :cc, :], fsrc.ap()[base:base + nrow],
                                gt[:, :c16], rows, rows, IN,
                                single_packet=False)
                            xb16 = xcp.tile([P, CC, IN], BF16, tag="xb16")
                            nc.scalar.copy(out=xb16[:, :cc, :],
                                           in_=xb[:, :cc, :])
                            # chunk -> (tile, first, last)
                            chmeta = []
                            for t, n in cl['segs']:
                                for k in range(n):
                                    chmeta.append((t, k == 0, k == n - 1))
                            ps = None
                            for ob in range(0, cc, cfg.OHC):
                                obc = min(cfg.OHC, cc - ob)
                                oh = ohp.tile([P, cfg.OHC, P], BF16, tag="oh")
                                nc.vector.tensor_tensor(
                                    out=oh[:, :obc, :],
                                    in0=dl[:, ob:ob + obc, :].to_broadcast(
                                        (P, obc, P)),
                                    in1=iota[:, 0:1, :].to_broadcast(
                                        (P, obc, P)),
                                    op=ALU.is_equal)
                                for j in range(obc):
                                    t, first, last = chmeta[ob + j]
                                    if first:
                                        ps = psp.tile([P, IN], F32,
                                                      space="PSUM", tag="ps")
                                    nc.tensor.matmul(
                                        out=ps[:], lhsT=oh[:, j, :],
                                        rhs=xb16[:, ob + j, :],
                                        start=first, stop=last)
                                    if last:
                                        ys = yacc[:, (r * TP + t) * IN:
                                                  (r * TP + t + 1) * IN]
                                        nc.vector.tensor_tensor(
                                            out=ys, in0=ys, in1=ps[:],
                                            op=ALU.add)
                    # ---------------- transform ----------------
                    for t in range(TP):
                        msgp = p3p.tile([P, cfg.MSG], F32, space="PSUM",
                                        tag="msgp")
                        for r in range(R):
                            ys = yacc[:, (r * TP + t) * IN:(r * TP + t + 1) * IN]
                            pst = p3p.tile([IN, P], BF16, space="PSUM", tag="pst")
                            nc.tensor.transpose(out=pst[:], in_=ys,
                                                identity=ident16[:])
                            yT = p3.tile([IN, P], BF16, tag="yT")
                            if r % 2 == 0:
                                nc.scalar.copy(out=yT[:], in_=pst[:])
                            else:
                                nc.vector.tensor_copy(out=yT[:], in_=pst[:])
                            nc.tensor.matmul(
                                out=msgp[:, r * 16:(r + 1) * 16],
                                lhsT=yT[:], rhs=w64b[:, r, :],
                                start=True, stop=True)
                        msg = p3.tile([P, cfg.MSG], F32, tag="msg")
                        nc.scalar.activation(
                            out=msg[:], in_=msgp[:],
                            func=ACTF.Relu,
                            scale=cisb[:, d * TP + t: d * TP + t + 1])
                        psmT = p3p.tile([cfg.MSG, P], F32, space="PSUM",
                                        tag="psmT")
                        nc.tensor.transpose(out=psmT[:], in_=msg[:],
                                            identity=ident[:])
                        msgT = p3.tile([cfg.MSG, P], F32, tag="msgT")
                        nc.scalar.copy(out=msgT[:], in_=psmT[:])
                        fcp = p3p.tile([P, 64], F32, space="PSUM", tag="fcp")
                        nc.tensor.matmul(
                            out=fcp[:], lhsT=msgT[:], rhs=fcwT[:],
                            start=True, stop=True)
                        osb = p3.tile([P, 64], F32, tag="osb")
                        nc.vector.tensor_tensor(out=osb[:], in0=fcp[:],
                                                in1=fcb[:], op=ALU.add)
                        dst = m_out if d == 0 else u_out
                        nc.sync.dma_start(
                            out=dst.ap()[t * P:(t + 1) * P], in_=osb[:])
    nc.compile()
    return nc


# ----------------------------------------------------------------- kernel

def make_in_maps(cfg, gidx, dloc, inputs):
    ins = {k: np.asarray(v) for k, v in inputs.items()}
    iota = np.tile(np.arange(P, dtype=np.float32), (P, 1))
    base = dict(
        ufeat=ins['ufeat'], ifeat=ins['ifeat'],
        cj_u=ins['cj_user'], cj_m=ins['cj_movie'],
        attT=np.ascontiguousarray(ins['att'].T),
        basis2=ins['basis'].reshape(cfg.BAS, cfg.IN * 16).copy(),
        fc_w=ins['fc_w'], fc_b=ins['fc_b'].reshape(1, 64).copy(),
        iota128=iota,
    )
    in_maps = []
    for c in range(cfg.NCORES):
        ci = np.zeros((2 * cfg.TP * P, 1), np.float32)
        ci[:cfg.NSH] = ins['ci_movie'][c * cfg.NSH:(c + 1) * cfg.NSH]
        ci[cfg.TP * P:cfg.TP * P + cfg.NSH] = \
            ins['ci_user'][c * cfg.NSH:(c + 1) * cfg.NSH]
        in_maps.append({**base, 'ci_sh': ci, 'gidx': gidx[c], 'dloc': dloc[c]})
    return in_maps


def assemble(cfg, results):
    u = np.concatenate([results[c]['u_out'][:cfg.NSH]
                        for c in range(cfg.NCORES)])
    m = np.concatenate([results[c]['m_out'][:cfg.NSH]
                        for c in range(cfg.NCORES)])
    return u, m


def kernel(**inputs):
    from concourse import bass_utils
    cfg = Cfg()
    streams, gidx, dloc = build_plan(cfg, inputs['edge_user'],
                                     inputs['edge_movie'])
    nc = build_program(cfg, streams, gidx[0].shape[1], dloc[0].shape[1],
                       cfg.NCORES)
    in_maps = make_in_maps(cfg, gidx, dloc, inputs)
    res = bass_utils.run_bass_kernel_spmd(nc, in_maps,
                                          core_ids=list(range(cfg.NCORES)))
    return assemble(cfg, res.results)
